# revision 2
# baseline (speedup 1.0000x reference)
"""GCN feature extractor on 8 Trainium2 NeuronCores.

Distribution: nodes are block-sharded over 8 cores (12500 each, padded to
12544 = 98*128). Within each core, nodes are ordered by in-degree so that
128-node destination tiles have near-uniform neighbor counts. Per GCN layer
each core computes its slice of dinv*(h@W), an AllGather replicates the full
table, and per-tile neighbor features are fetched with indirect DMA gathers
(one 256B row per descriptor) and summed on the vector engine. BatchNorm
statistics are combined with a tiny AllReduce. Mean+max graph pooling reuses
the same gather machinery against the final node features; each core pools 32
graphs and the [256,64] result is assembled with a final AllGather.
"""

import numpy as np

N = 100000
E = 1600000
D = 64
G = 256
NC = 8
NPC_RAW = 12500
NPC = 12544          # 98 tiles of 128
NT = NPC // 128      # 98
TR = NC * NPC        # 100352 table rows
EPS = 1e-5
ZPAD = NPC_RAW       # table row 12500 (core 0 phantom) is always zero
GPC = G // NC        # 32 graphs per core
DEG_INF = np.float32(1e38)


def _preprocess(edge_index, batch):
    src = np.asarray(edge_index[0], dtype=np.int64)
    dst = np.asarray(edge_index[1], dtype=np.int64)
    batch = np.asarray(batch, dtype=np.int64)

    core = np.arange(N, dtype=np.int64) // NPC_RAW          # exact: 99999//12500 = 7
    indeg = np.bincount(dst, minlength=N).astype(np.int64)

    # table id: group by core, sort by in-degree descending within core
    order = np.lexsort((np.arange(N), -indeg, core))
    core_sorted = core[order]
    block_start = np.searchsorted(core_sorted, np.arange(NC))
    within = np.arange(N, dtype=np.int64) - block_start[core_sorted]
    tid = np.empty(N, dtype=np.int64)
    tid[order] = core_sorted * NPC + within

    src_t = tid[src]
    dst_t = tid[dst]

    # per-tile max degree across cores
    indeg_row = np.zeros(TR, dtype=np.int64)
    indeg_row[tid] = indeg
    deg_tiles = indeg_row.reshape(NC, NT, 128)
    K_t = deg_tiles.max(axis=(0, 2)).astype(np.int64)       # [NT]
    coloff = np.concatenate([[0], np.cumsum(K_t)])          # [NT+1]
    SK = int(coloff[-1])

    # slot assignment: edges sorted by destination row, rank within segment
    eorder = np.argsort(dst_t, kind="stable")
    ds = dst_t[eorder]
    ss = src_t[eorder]
    seg_start = np.searchsorted(ds, np.arange(TR))
    rank = np.arange(E, dtype=np.int64) - seg_start[ds]
    c_of = ds // NPC
    within_row = ds % NPC
    tile_of = within_row // 128
    p_of = within_row % 128
    col = coloff[tile_of] + rank
    idx_np = np.full((NC, 128, SK), ZPAD, dtype=np.int32)
    idx_np[c_of, p_of, col] = ss.astype(np.int32)

    # degree (with self loop) per row, phantoms get DEG_INF so dinv ~ 0
    deg_f = np.full(TR, DEG_INF, dtype=np.float32)
    real = (np.arange(TR) % NPC) < NPC_RAW
    deg_f[real] = (indeg_row[real] + 1).astype(np.float32)
    deg_f = deg_f.reshape(NC, NT, 128).transpose(0, 2, 1).copy()  # [NC,128,NT]

    # pooling: graphs blocked by id; members are contiguous in original order
    cnt = np.bincount(batch, minlength=G).astype(np.int64)
    gstart = np.concatenate([[0], np.cumsum(cnt)])
    KP = int(np.ceil(cnt.max() / 4)) if cnt.max() > 0 else 1
    idxP = np.full((NC, 128, KP), ZPAD, dtype=np.int32)
    npad = np.full((NC, 128, 1), float(KP), dtype=np.float32)
    member0 = np.full((NC, 128, 1), ZPAD, dtype=np.int32)
    cntinv = np.zeros((NC, GPC), dtype=np.float32)
    for g in range(G):
        c, gl = g // GPC, g % GPC
        mem = tid[gstart[g]:gstart[g + 1]]
        if len(mem) == 0:
            continue
        cntinv[c, gl] = 1.0 / len(mem)
        m0 = np.int32(mem[0])
        for q in range(4):
            p = 4 * gl + q
            mq = mem[q::4]
            idxP[c, p, :] = m0
            idxP[c, p, :len(mq)] = mq
            npad[c, p, 0] = KP - len(mq)
            member0[c, p, 0] = m0
    cntinv_fm = np.repeat(cntinv[:, None, :], D, axis=1).copy()  # [NC, 64, GPC]

    return dict(
        idx=idx_np, K_t=K_t, coloff=coloff, SK=SK, tid=tid, deg_f=deg_f,
        idxP=idxP, npad=npad, member0=member0, cntinv_fm=cntinv_fm, KP=KP,
    )


def _numpy_model(x, prep, Ws, gs, bes):
    """float32 mirror of the device algorithm (validation only)."""
    tid = prep["tid"]
    xp = np.zeros((TR, D), dtype=np.float32)
    xp[tid] = x
    h_fm = np.stack([xp[c * NPC:(c + 1) * NPC].T for c in range(NC)])  # [NC,64,NPC]
    deg = prep["deg_f"]                                   # [NC,128,NT]
    dinv = np.sqrt(np.float32(1.0) / deg).astype(np.float32)
    dinv_flat = dinv.transpose(0, 2, 1).reshape(NC, NPC)  # row-major per core
    idx_np, K_t, coloff = prep["idx"], prep["K_t"], prep["coloff"]

    for l in range(3):
        W, g_, be = Ws[l], gs[l], bes[l]
        table = np.zeros((TR, D), dtype=np.float32)
        hhat = np.zeros((NC, NPC, D), dtype=np.float32)
        for c in range(NC):
            t_fm = (W.T.astype(np.float32) @ h_fm[c]).astype(np.float32)
            hh = (t_fm.T * dinv_flat[c][:, None]).astype(np.float32)
            hhat[c] = hh
            table[c * NPC:(c + 1) * NPC] = hh
        y_fm = np.zeros((NC, D, NPC), dtype=np.float32)
        ssum = np.zeros((NC, D), dtype=np.float32)
        sq = np.zeros((NC, D), dtype=np.float32)
        for c in range(NC):
            for t in range(NT):
                K = int(K_t[t])
                sl = slice(t * 128, (t + 1) * 128)
                msum = np.zeros((128, D), dtype=np.float32)
                if K:
                    slab = table[idx_np[c, :, coloff[t]:coloff[t] + K]]  # [128,K,64]
                    msum = slab.sum(axis=1, dtype=np.float32)
                acc = ((msum + hhat[c][sl]) * dinv[c, :, t][:, None]).astype(np.float32)
                y_fm[c][:, sl] = acc.T
            ssum[c] = y_fm[c].sum(axis=1, dtype=np.float32)
            sq[c] = (y_fm[c].astype(np.float32) ** 2).sum(axis=1, dtype=np.float32)
        S = ssum.sum(axis=0, dtype=np.float32)
        Q = sq.sum(axis=0, dtype=np.float32)
        mean = (S / np.float32(N)).astype(np.float32)
        var = (Q / np.float32(N) - mean * mean).astype(np.float32)
        rstd = np.sqrt(np.float32(1.0) / (var + np.float32(EPS))).astype(np.float32)
        scale = (g_ * rstd).astype(np.float32)
        shift = (be - mean * scale).astype(np.float32)
        for c in range(NC):
            h = (y_fm[c] * scale[:, None] + shift[:, None]).astype(np.float32)
            if l < 2:
                h = np.maximum(h, 0)
            h_fm[c] = h

    # pooling from full h3 table
    table = np.zeros((TR, D), dtype=np.float32)
    for c in range(NC):
        table[c * NPC:(c + 1) * NPC] = h_fm[c].T
    idxP, npad, member0, cntinv_fm = (
        prep["idxP"], prep["npad"], prep["member0"], prep["cntinv_fm"])
    out = np.zeros((G, D), dtype=np.float32)
    for c in range(NC):
        slab = table[idxP[c]]                      # [128, KP, 64]
        ssum = slab.sum(axis=1, dtype=np.float32)  # [128, 64]
        smax = slab.max(axis=1)
        m0row = table[member0[c, :, 0]]            # [128, 64]
        ssum = ssum - npad[c] * m0row
        q = ssum.reshape(GPC, 4, D).sum(axis=1, dtype=np.float32)   # [32, 64]
        m = smax.reshape(GPC, 4, D).max(axis=1)
        mean = q * cntinv_fm[c, 0][:, None]
        out[c * GPC:(c + 1) * GPC] = mean + m
    return out


def kernel(**inputs):
    x = np.asarray(inputs["x"], dtype=np.float32)
    prep = _preprocess(inputs["edge_index"], inputs["batch"])
    Ws = [np.asarray(inputs[f"W{i+1}"], dtype=np.float32) for i in range(3)]
    gs = [np.asarray(inputs[f"g{i+1}"], dtype=np.float32) for i in range(3)]
    bes = [np.asarray(inputs[f"be{i+1}"], dtype=np.float32) for i in range(3)]
    import os
    if os.environ.get("GCN_NUMPY_MODEL"):
        return _numpy_model(x, prep, Ws, gs, bes)
    return _run_device(x, prep, Ws, gs, bes)


HNPC = NPC // 2      # 6272 columns per half of the split feature-major layout
HT = NT // 2         # 49 tiles per half

_DEVICE_CACHE = {}


def _build_device(K_t, coloff, SK, KP, reps=1, agg_mode="full"):
    import concourse.bacc as bacc
    import concourse.bass as bass
    import concourse.tile as tile
    import concourse.mybir as mybir
    from concourse.masks import make_identity

    fp32 = mybir.dt.float32
    i32 = mybir.dt.int32
    KMAX = int(max(int(K_t.max()), 1))

    nc = bacc.Bacc("TRN2", target_bir_lowering=False, debug=False, num_devices=NC)

    x_in = nc.dram_tensor("x_in", [128, HNPC], fp32, kind="ExternalInput")
    w_in = nc.dram_tensor("w_in", [3 * 64, 64], fp32, kind="ExternalInput")
    bn_in = nc.dram_tensor("bn_in", [6 * 64, 1], fp32, kind="ExternalInput")
    deg_in = nc.dram_tensor("deg_in", [128, NT], fp32, kind="ExternalInput")
    idx_in = nc.dram_tensor("idx_in", [128, SK], i32, kind="ExternalInput")
    idxp_in = nc.dram_tensor("idxp_in", [128, KP], i32, kind="ExternalInput")
    mem0_in = nc.dram_tensor("mem0_in", [128, 1], i32, kind="ExternalInput")
    npad_in = nc.dram_tensor("npad_in", [128, 1], fp32, kind="ExternalInput")
    phm_in = nc.dram_tensor("phm_in", [128, 1], fp32, kind="ExternalInput")
    cntinv_in = nc.dram_tensor("cntinv_in", [64, GPC], fp32, kind="ExternalInput")
    out_ext = nc.dram_tensor("out", [G, D], fp32, kind="ExternalOutput")

    slice_d = nc.dram_tensor("slice_d", [NPC, D], fp32)
    table_d = nc.dram_tensor("table_d", [TR, D], fp32)
    stats_i = nc.dram_tensor("stats_i", [64, 2], fp32)
    stats_o = nc.dram_tensor("stats_o", [64, 2], fp32)
    oslice_d = nc.dram_tensor("oslice_d", [GPC, D], fp32)
    ofull_d = nc.dram_tensor("ofull_d", [G, D], fp32)

    RG = [list(range(NC))]
    INVN = 1.0 / float(N)

    with tile.TileContext(nc) as tc:
        with (
            tc.tile_pool(name="cp", bufs=1) as cp,
            tc.tile_pool(name="hp", bufs=2) as hp,
            tc.tile_pool(name="sp", bufs=3) as sp,
            tc.tile_pool(name="sm", bufs=4) as sm,
            tc.tile_pool(name="slb", bufs=4) as slb,
            tc.tile_pool(name="pl", bufs=1) as pl,
            tc.tile_pool(name="ps", bufs=2, space="PSUM") as ps,
        ):
            ident = cp.tile([128, 128], fp32, tag="ident")
            make_identity(nc, ident[:])

            deg_sb = cp.tile([128, NT], fp32, tag="deg")
            nc.sync.dma_start(out=deg_sb[:], in_=deg_in[:])
            dinv = cp.tile([128, NT], fp32, tag="dinv")
            nc.vector.reciprocal(out=dinv[:], in_=deg_sb[:])
            nc.scalar.activation(out=dinv[:], in_=dinv[:],
                                 func=mybir.ActivationFunctionType.Sqrt)

            idx_sb = cp.tile([128, SK], i32, tag="idx")
            nc.sync.dma_start(out=idx_sb[:], in_=idx_in[:])

            w_sb = []
            bn_sb = []
            def emit_body():
                h_cur = hp.tile([128, HNPC], fp32, tag="h")
                nc.sync.dma_start(out=h_cur[:], in_=x_in[:])
                for l in range(3):
                    wt = cp.tile([128, 64], fp32, tag=f"w{l}")
                    nc.sync.dma_start(out=wt[0:64, :], in_=w_in[l * 64:(l + 1) * 64, :])
                    nc.sync.dma_start(out=wt[64:128, :], in_=w_in[l * 64:(l + 1) * 64, :])
                    w_sb.append(wt)
                    bt = cp.tile([64, 2], fp32, tag=f"bn{l}")
                    nc.sync.dma_start(out=bt[:, 0:1], in_=bn_in[(2 * l) * 64:(2 * l + 1) * 64, :])
                    nc.sync.dma_start(out=bt[:, 1:2], in_=bn_in[(2 * l + 1) * 64:(2 * l + 2) * 64, :])
                    bn_sb.append(bt)

                hhat = cp.tile([128, NT * 64], fp32, tag="hhat")
                yst = cp.tile([128, HNPC], fp32, tag="yst")

                idxp_sb = cp.tile([128, KP], i32, tag="idxp")
                nc.sync.dma_start(out=idxp_sb[:], in_=idxp_in[:])
                mem0_sb = cp.tile([128, 1], i32, tag="mem0")
                nc.sync.dma_start(out=mem0_sb[:], in_=mem0_in[:])
                npad_sb = cp.tile([128, 1], fp32, tag="npad")
                nc.sync.dma_start(out=npad_sb[:], in_=npad_in[:])
                cinv_sb = cp.tile([64, GPC], fp32, tag="cinv")
                nc.sync.dma_start(out=cinv_sb[:], in_=cntinv_in[:])
                phm_sb = cp.tile([128, 1], fp32, tag="phm")
                nc.sync.dma_start(out=phm_sb[:], in_=phm_in[:])

                slice_v = slice_d[:].rearrange("(t p) d -> p t d", p=128)

                def emit_table_write_and_gather(src_nm):
                    # src_nm: [128, NT*64] staging of this core's table slice rows
                    nc.sync.dma_start(out=slice_v, in_=src_nm[:])
                    nc.gpsimd.collective_compute(
                        "AllGather", mybir.AluOpType.bypass, replica_groups=RG,
                        ins=[slice_d[:].opt()], outs=[table_d[:].opt()],
                    )

                for l in range(3):
                    # ---- GEMM + dinv scale -> hhat staging (table slice) ----
                    for t in range(NT):
                        half, c = divmod(t, HT)
                        pb = 64 * half
                        pt = ps.tile([64, 128], fp32, tag="pt", space="PSUM")
                        nc.tensor.matmul(
                            out=pt[:], lhsT=w_sb[l][pb:pb + 64, :],
                            rhs=h_cur[pb:pb + 64, c * 128:(c + 1) * 128],
                            start=True, stop=True,
                        )
                        stg = sp.tile([64, 128], fp32, tag="stg")
                        nc.vector.tensor_copy(out=stg[:], in_=pt[:])
                        ptr = ps.tile([128, 64], fp32, tag="ptr", space="PSUM")
                        nc.tensor.transpose(out=ptr[:], in_=stg[:], identity=ident[0:64, 0:64])
                        nc.vector.tensor_scalar_mul(
                            out=hhat[:, t * 64:(t + 1) * 64], in0=ptr[:],
                            scalar1=dinv[:, t:t + 1],
                        )
                    emit_table_write_and_gather(hhat)

                    # ---- gather + aggregate -> yst ----
                    def agg_loop():
                      for t in range(NT):
                          half, c = divmod(t, HT)
                          pb = 64 * half
                          K = int(K_t[t])
                          if K > 0:
                              slab = slb.tile([128, KMAX * 64], fp32, tag="slab")
                              for k in range(K):
                                  col = int(coloff[t]) + k
                                  nc.gpsimd.indirect_dma_start(
                                      out=slab[:, k * 64:(k + 1) * 64],
                                      out_offset=None,
                                      in_=table_d[:],
                                      in_offset=bass.IndirectOffsetOnAxis(
                                          ap=idx_sb[:, col:col + 1], axis=0),
                                  )
                              n = K
                              while n > 1:
                                  hl = n // 2
                                  if n % 2:
                                      nc.vector.tensor_tensor(
                                          out=slab[:, 0:64], in0=slab[:, 0:64],
                                          in1=slab[:, (n - 1) * 64:n * 64],
                                          op=mybir.AluOpType.add)
                                  nc.vector.tensor_tensor(
                                      out=slab[:, 0:hl * 64], in0=slab[:, 0:hl * 64],
                                      in1=slab[:, hl * 64:2 * hl * 64],
                                      op=mybir.AluOpType.add)
                                  n = hl
                              msum_ap = slab[:, 0:64]
                          else:
                              msum = sm.tile([128, 64], fp32, tag="msum")
                              nc.gpsimd.memset(msum[:], 0.0)
                              msum_ap = msum[:]
                          acc = sm.tile([128, 64], fp32, tag="acc")
                          nc.vector.tensor_tensor(
                              out=acc[:], in0=msum_ap, in1=hhat[:, t * 64:(t + 1) * 64],
                              op=mybir.AluOpType.add)
                          ynm = sm.tile([128, 64], fp32, tag="ynm")
                          nc.vector.tensor_scalar_mul(out=ynm[:], in0=acc[:],
                                                      scalar1=dinv[:, t:t + 1])
                          pyt = ps.tile([64, 128], fp32, tag="pyt", space="PSUM")
                          nc.tensor.transpose(out=pyt[:], in_=ynm[:], identity=ident[:, 0:128])
                          nc.vector.tensor_copy(
                              out=yst[pb:pb + 64, c * 128:(c + 1) * 128], in_=pyt[:])

                    def gathers_only():
                        for t in range(NT):
                            K = int(K_t[t])
                            if K == 0:
                                continue
                            slab = slb.tile([128, KMAX * 64], fp32, tag="slab")
                            for k in range(K):
                                col = int(coloff[t]) + k
                                nc.gpsimd.indirect_dma_start(
                                    out=slab[:, k * 64:(k + 1) * 64],
                                    out_offset=None,
                                    in_=table_d[:],
                                    in_offset=bass.IndirectOffsetOnAxis(
                                        ap=idx_sb[:, col:col + 1], axis=0),
                                )

                    if reps == 1:
                        agg_loop()
                    else:
                        body_fn = gathers_only if agg_mode == "gonly" else agg_loop
                        with tc.For_i(0, reps, 1) as _r:
                            body_fn()
                        if agg_mode == "gonly":
                            agg_loop()

                    # ---- BN stats (sum, sumsq) over both halves ----
                    stt = sm.tile([128, 2], fp32, tag="stt")
                    sqs = sm.tile([64, 1024], fp32, tag="sqs")
                    parts = []
                    for half in range(2):
                        pb = 64 * half
                        srow = sm.tile([128, 1], fp32, tag=f"srow{half}")
                        nc.vector.reduce_sum(out=srow[0:64, :], in_=yst[pb:pb + 64, :],
                                             axis=mybir.AxisListType.X)
                        qacc = sm.tile([128, 8], fp32, tag=f"qacc{half}")
                        nchunk = (HNPC + 1023) // 1024
                        for j in range(nchunk):
                            lo = j * 1024
                            hi = min(lo + 1024, HNPC)
                            nc.vector.tensor_tensor(
                                out=sqs[:, 0:hi - lo], in0=yst[pb:pb + 64, lo:hi],
                                in1=yst[pb:pb + 64, lo:hi], op=mybir.AluOpType.mult)
                            nc.vector.reduce_sum(
                                out=qacc[0:64, j:j + 1], in_=sqs[:, 0:hi - lo],
                                axis=mybir.AxisListType.X)
                        qsum = sm.tile([128, 1], fp32, tag=f"qsum{half}")
                        nc.vector.reduce_sum(out=qsum[0:64, :], in_=qacc[0:64, 0:nchunk],
                                             axis=mybir.AxisListType.X)
                        parts.append((srow, qsum))
                    nc.vector.tensor_tensor(out=stt[0:64, 0:1], in0=parts[0][0][0:64, :],
                                            in1=parts[1][0][0:64, :], op=mybir.AluOpType.add)
                    nc.vector.tensor_tensor(out=stt[0:64, 1:2], in0=parts[0][1][0:64, :],
                                            in1=parts[1][1][0:64, :], op=mybir.AluOpType.add)
                    nc.sync.dma_start(out=stats_i[:], in_=stt[0:64, :])
                    nc.gpsimd.collective_compute(
                        "AllReduce", mybir.AluOpType.add, replica_groups=RG,
                        ins=[stats_i[:].opt()], outs=[stats_o[:].opt()],
                    )
                    stin = sm.tile([64, 2], fp32, tag="stin")
                    nc.sync.dma_start(out=stin[:], in_=stats_o[:])

                    # ---- BN coefficients ----
                    co = sm.tile([64, 8], fp32, tag="co")
                    mean, ex2, m2, var, rec, rstd = (co[:, i:i + 1] for i in range(6))
                    nc.vector.tensor_scalar_mul(out=mean, in0=stin[:, 0:1], scalar1=INVN)
                    nc.vector.tensor_scalar_mul(out=ex2, in0=stin[:, 1:2], scalar1=INVN)
                    nc.vector.tensor_tensor(out=m2, in0=mean, in1=mean, op=mybir.AluOpType.mult)
                    nc.vector.tensor_tensor(out=var, in0=ex2, in1=m2, op=mybir.AluOpType.subtract)
                    nc.vector.tensor_scalar_add(out=var, in0=var, scalar1=float(EPS))
                    nc.vector.reciprocal(out=rec, in_=var)
                    nc.scalar.activation(out=rstd, in_=rec, func=mybir.ActivationFunctionType.Sqrt)
                    scsh = sm.tile([128, 2], fp32, tag="scsh")
                    nc.vector.tensor_tensor(out=scsh[0:64, 0:1], in0=bn_sb[l][:, 0:1],
                                            in1=rstd, op=mybir.AluOpType.mult)
                    ms = co[:, 6:7]
                    nc.vector.tensor_tensor(out=ms, in0=mean, in1=scsh[0:64, 0:1],
                                            op=mybir.AluOpType.mult)
                    nc.vector.tensor_tensor(out=scsh[0:64, 1:2], in0=bn_sb[l][:, 1:2],
                                            in1=ms, op=mybir.AluOpType.subtract)
                    nc.vector.tensor_copy(out=scsh[64:128, :], in_=scsh[0:64, :])

                    # ---- BN apply (+ReLU) -> next h ----
                    h_nxt = hp.tile([128, HNPC], fp32, tag="h")
                    for half in range(2):
                        pb = 64 * half
                        if l < 2:
                            nc.scalar.activation(
                                out=h_nxt[pb:pb + 64, :], in_=yst[pb:pb + 64, :],
                                func=mybir.ActivationFunctionType.Relu,
                                bias=scsh[pb:pb + 64, 1:2], scale=scsh[pb:pb + 64, 0:1])
                        else:
                            nc.vector.tensor_scalar(
                                out=h_nxt[pb:pb + 64, :], in0=yst[pb:pb + 64, :],
                                scalar1=scsh[pb:pb + 64, 0:1], scalar2=scsh[pb:pb + 64, 1:2],
                                op0=mybir.AluOpType.mult, op1=mybir.AluOpType.add)
                    h_cur = h_nxt

                # ---- h3 -> table ----
                for t in range(NT):
                    half, c = divmod(t, HT)
                    pb = 64 * half
                    ph = ps.tile([128, 64], fp32, tag="ptr", space="PSUM")
                    nc.tensor.transpose(out=ph[:], in_=h_cur[pb:pb + 64, c * 128:(c + 1) * 128],
                                        identity=ident[pb:pb + 64, pb:pb + 64])
                    nc.vector.tensor_copy(out=hhat[:, t * 64:(t + 1) * 64], in_=ph[:])
                nc.vector.tensor_scalar_mul(
                    out=hhat[:, (NT - 1) * 64:NT * 64],
                    in0=hhat[:, (NT - 1) * 64:NT * 64], scalar1=phm_sb[:, 0:1])
                emit_table_write_and_gather(hhat)

                # ---- pooling ----
                slabp = pl.tile([128, KP * 64], fp32, tag="pslab")
                for k in range(KP):
                    nc.gpsimd.indirect_dma_start(
                        out=slabp[:, k * 64:(k + 1) * 64], out_offset=None,
                        in_=table_d[:],
                        in_offset=bass.IndirectOffsetOnAxis(ap=idxp_sb[:, k:k + 1], axis=0),
                    )
                m0row = sm.tile([128, 64], fp32, tag="m0row")
                nc.gpsimd.indirect_dma_start(
                    out=m0row[:], out_offset=None, in_=table_d[:],
                    in_offset=bass.IndirectOffsetOnAxis(ap=mem0_sb[:, 0:1], axis=0),
                )
                pv = slabp[:].rearrange("p (k d) -> p d k", k=KP)
                ssum = sm.tile([128, 64], fp32, tag="ssum")
                nc.vector.reduce_sum(out=ssum[:], in_=pv, axis=mybir.AxisListType.X)
                smax = sm.tile([128, 64], fp32, tag="smax")
                nc.vector.reduce_max(out=smax[:], in_=pv, axis=mybir.AxisListType.X)
                corr = sm.tile([128, 64], fp32, tag="corr")
                nc.vector.tensor_scalar_mul(out=corr[:], in0=m0row[:], scalar1=npad_sb[:, 0:1])
                nc.vector.tensor_tensor(out=ssum[:], in0=ssum[:], in1=corr[:],
                                        op=mybir.AluOpType.subtract)

                def to_fm(src, tg):
                    p = ps.tile([64, 128], fp32, tag="pyt", space="PSUM")
                    nc.tensor.transpose(out=p[:], in_=src[:], identity=ident[:, 0:128])
                    t = sm.tile([64, 128], fp32, tag="fm" + tg)
                    nc.vector.tensor_copy(out=t[:], in_=p[:])
                    return t

                sfm = to_fm(ssum, "s")
                mfm = to_fm(smax, "m")

                def qcombine(t, op, tg):
                    v = t[:].rearrange("f (g q) -> f q g", q=4)
                    a = sm.tile([64, GPC], fp32, tag="qa" + tg)
                    b = sm.tile([64, GPC], fp32, tag="qb" + tg)
                    nc.vector.tensor_tensor(out=a[:], in0=v[:, 0, :], in1=v[:, 1, :], op=op)
                    nc.vector.tensor_tensor(out=b[:], in0=v[:, 2, :], in1=v[:, 3, :], op=op)
                    nc.vector.tensor_tensor(out=a[:], in0=a[:], in1=b[:], op=op)
                    return a

                s32 = qcombine(sfm, mybir.AluOpType.add, "s")
                m32 = qcombine(mfm, mybir.AluOpType.max, "m")
                outfm = sm.tile([64, GPC], fp32, tag="outfm")
                nc.vector.tensor_tensor(out=outfm[:], in0=s32[:], in1=cinv_sb[:],
                                        op=mybir.AluOpType.mult)
                nc.vector.tensor_tensor(out=outfm[:], in0=outfm[:], in1=m32[:],
                                        op=mybir.AluOpType.add)
                po = ps.tile([GPC, 64], fp32, tag="ptr", space="PSUM")
                nc.tensor.transpose(out=po[:], in_=outfm[:], identity=ident[0:64, 0:64])
                onm = sm.tile([GPC, 64], fp32, tag="onm")
                nc.vector.tensor_copy(out=onm[:], in_=po[:])
                nc.sync.dma_start(out=oslice_d[:], in_=onm[:])
                nc.gpsimd.collective_compute(
                    "AllGather", mybir.AluOpType.bypass, replica_groups=RG,
                    ins=[oslice_d[:].opt()], outs=[ofull_d[:].opt()],
                )
                for half in range(2):
                    ot = sm.tile([128, 64], fp32, tag="ot")
                    nc.sync.dma_start(out=ot[:], in_=ofull_d[half * 128:(half + 1) * 128, :])
                    nc.sync.dma_start(out=out_ext[half * 128:(half + 1) * 128, :], in_=ot[:])


            emit_body()

    nc.compile()
    return nc


_PHMASK = (np.arange(12416, 12544)[:, None] < NPC_RAW).astype(np.float32)


def _make_inmaps(x, prep, Ws, gs, bes):
    tid = prep["tid"]
    xp = np.zeros((TR, D), dtype=np.float32)
    xp[tid] = x
    w_np = np.concatenate(Ws, axis=0).astype(np.float32)          # [192, 64]
    bn_np = np.zeros((6 * 64, 1), dtype=np.float32)
    for l in range(3):
        bn_np[(2 * l) * 64:(2 * l + 1) * 64, 0] = gs[l]
        bn_np[(2 * l + 1) * 64:(2 * l + 2) * 64, 0] = bes[l]
    in_maps = []
    for c in range(NC):
        sl = xp[c * NPC:(c + 1) * NPC]                            # [NPC, 64]
        xs = np.zeros((128, HNPC), dtype=np.float32)
        xs[0:64, :] = sl[:HNPC].T
        xs[64:128, :] = sl[HNPC:].T
        in_maps.append({
            "x_in": xs,
            "w_in": w_np,
            "bn_in": bn_np,
            "deg_in": prep["deg_f"][c],
            "idx_in": prep["idx"][c],
            "idxp_in": prep["idxP"][c],
            "mem0_in": prep["member0"][c],
            "npad_in": prep["npad"][c],
            "phm_in": _PHMASK,
            "cntinv_in": prep["cntinv_fm"][c],
        })
    return in_maps


def _run_device(x, prep, Ws, gs, bes):
    from concourse.bass_utils import run_bass_kernel_spmd

    import os
    reps = int(os.environ.get("GCN_REPS", "1"))
    agg_mode = os.environ.get("GCN_AGG_MODE", "full")
    key = (prep["SK"], prep["KP"], reps, agg_mode, tuple(int(k) for k in prep["K_t"]))
    if key not in _DEVICE_CACHE:
        _DEVICE_CACHE[key] = _build_device(
            prep["K_t"], prep["coloff"], prep["SK"], prep["KP"], reps=reps,
            agg_mode=agg_mode)
    nc = _DEVICE_CACHE[key]
    in_maps = _make_inmaps(x, prep, Ws, gs, bes)
    trace = bool(os.environ.get("GCN_TRACE"))
    kw = {}
    if trace:
        kw["trace"] = True
        td = os.environ.get("GCN_TRACE_DIR")
        if td:
            os.makedirs(td, exist_ok=True)
            kw["tmpdir"] = td
    res = run_bass_kernel_spmd(nc, in_maps, core_ids=list(range(NC)), **kw)
    global _LAST_RES
    _LAST_RES = res
    return np.asarray(res.results[0]["out"], dtype=np.float32)

